# revision 1
# baseline (speedup 1.0000x reference)
"""Graphformer layer (full multi-head attention) on 8 trn2 NeuronCores.

Sharding: one head per core (tensor parallel over the 8 heads).
Each core computes, for its head h:
    Q_h = x Wq_h^T, K_h = x Wk_h^T, V_h = x Wv_h^T          (4096, 64)
    S_h = Q_h K_h^T / 8;  P_h = softmax(S_h)                 (4096, 4096)
    y_core = (P_h V_h) Wo_h^T                                (4096, 64)
Full output = sum over cores + bo.

On-chip formulation avoids all big transposes:
  - host passes x^T, Wq^T, Wk^T, Wv^T so the contracted feature dim is on
    partitions.
  - scores are computed transposed (S^T tiles: keys on partitions, queries
    on the free dim); softmax denominators come from an appended ones
    column on V (row 64 of the O^T accumulator).
  - exp() is applied without max-subtraction: |S/8| < ~3 for these inputs,
    exact for fp32.
  - normalization by the softmax denominator folds into the final output
    projection via an augmented (65,65) Wo^T with a 1 in the corner:
    column 64 of the Y tile is the per-row denominator.
"""

from contextlib import ExitStack

import numpy as np

import concourse.bass as bass
import concourse.bacc as bacc
import concourse.mybir as mybir
from concourse.tile import TileContext

N = 4096
C = 512  # input feature dim
D = 64  # head dim
Da = D + 1  # head dim + denominator column
HEADS = 8
P = 128
F32 = mybir.dt.float32


def build_nc(n=N, f=1024):
    """Build the single-core SPMD program. n = sequence length, f = query
    group width (exp granularity; f*4 bytes*2 buffers of PSUM for scores)."""
    nt = n // P  # number of key/value tiles
    ct = C // P  # contraction tiles for projections
    g_count = n // f  # query groups

    nc = bacc.Bacc()
    xT = nc.declare_dram_parameter("xT", [C, n], F32, isOutput=False)
    wqT = nc.declare_dram_parameter("wqT", [C, D], F32, isOutput=False)
    wkT = nc.declare_dram_parameter("wkT", [C, D], F32, isOutput=False)
    wvT = nc.declare_dram_parameter("wvT", [C, D], F32, isOutput=False)
    woT = nc.declare_dram_parameter("woT", [Da, Da], F32, isOutput=False)
    y = nc.declare_dram_parameter("y", [n, D], F32, isOutput=True)

    with TileContext(nc) as tc, ExitStack() as ctx:
        const = ctx.enter_context(tc.tile_pool(name="const", bufs=1))
        sb = ctx.enter_context(tc.tile_pool(name="sb", bufs=1))
        es_pool = ctx.enter_context(tc.tile_pool(name="es", bufs=3))
        ot_pool = ctx.enter_context(tc.tile_pool(name="ot", bufs=2))
        y_pool = ctx.enter_context(tc.tile_pool(name="yp", bufs=4))

        # ---- load inputs
        xt = []
        for c in range(ct):
            t = sb.tile([P, n], F32, tag=f"xt{c}")
            nc.sync.dma_start(out=t, in_=xT[c * P : (c + 1) * P, :])
            xt.append(t)
        w_sb = {}
        for name, dram in (("q", wqT), ("k", wkT), ("v", wvT)):
            t = const.tile([P, ct, D], F32, tag=f"w{name}")
            for c in range(ct):
                nc.sync.dma_start(out=t[:, c, :], in_=dram[c * P : (c + 1) * P, :])
            w_sb[name] = t
        wo_sb = const.tile([Da, Da], F32, tag="wo")
        nc.sync.dma_start(out=wo_sb, in_=woT[:, :])

        # ---- projections
        qT = sb.tile([D, n], F32, tag="qT")
        kT = sb.tile([D, n], F32, tag="kT")
        v_sb = sb.tile([P, nt, Da], F32, tag="v")
        with tc.tile_pool(name="psP", bufs=4, space="PSUM") as psP:
            for chunk in range(n // 512):
                for dst, w in ((qT, w_sb["q"]), (kT, w_sb["k"])):
                    pp = psP.tile([D, 512], F32, tag="pqk")
                    for c in range(ct):
                        nc.tensor.matmul(
                            pp,
                            w[:, c, :],
                            xt[c][:, chunk * 512 : (chunk + 1) * 512],
                            start=(c == 0),
                            stop=(c == ct - 1),
                        )
                    nc.vector.tensor_copy(
                        out=dst[:, chunk * 512 : (chunk + 1) * 512], in_=pp
                    )
            nc.vector.memset(v_sb[:, :, D:Da], 1.0)
            for mt in range(nt):
                pv = psP.tile([P, D], F32, tag="pv")
                for c in range(ct):
                    nc.tensor.matmul(
                        pv,
                        xt[c][:, mt * P : (mt + 1) * P],
                        w_sb["v"][:, c, :],
                        start=(c == 0),
                        stop=(c == ct - 1),
                    )
                nc.vector.tensor_copy(out=v_sb[:, mt, 0:D], in_=pv)

        # ---- attention + output projection, in query groups of f
        with (
            tc.tile_pool(name="psS", bufs=2, space="PSUM") as ps_s,
            tc.tile_pool(name="psO", bufs=1, space="PSUM") as ps_o,
            tc.tile_pool(name="psY", bufs=2, space="PSUM") as ps_y,
        ):
            for g in range(g_count):
                po = ps_o.tile([Da, f], F32, tag="O")
                for mt in range(nt):
                    ss = ps_s.tile([P, f], F32, tag="S")
                    for fc in range(f // 512):
                        nc.tensor.matmul(
                            ss[:, fc * 512 : (fc + 1) * 512],
                            kT[:, mt * P : (mt + 1) * P],
                            qT[:, g * f + fc * 512 : g * f + (fc + 1) * 512],
                            start=True,
                            stop=True,
                        )
                    es = es_pool.tile([P, f], F32, tag="es")
                    nc.scalar.activation(
                        out=es,
                        in_=ss,
                        func=mybir.ActivationFunctionType.Exp,
                        scale=0.125,
                    )
                    for fc in range(f // 512):
                        nc.tensor.matmul(
                            po[:, fc * 512 : (fc + 1) * 512],
                            v_sb[:, mt, :],
                            es[:, fc * 512 : (fc + 1) * 512],
                            start=(mt == 0),
                            stop=(mt == nt - 1),
                        )
                ot = ot_pool.tile([Da, f], F32, tag="ot")
                nc.vector.tensor_copy(out=ot, in_=po)
                for it in range(f // P):
                    py = ps_y.tile([P, Da], F32, tag="Y")
                    nc.tensor.matmul(
                        py,
                        ot[:, it * P : (it + 1) * P],
                        wo_sb,
                        start=True,
                        stop=True,
                    )
                    rec = y_pool.tile([P, 1], F32, tag="rec")
                    nc.vector.reciprocal(rec, py[:, D:Da])
                    ysb = y_pool.tile([P, D], F32, tag="ysb")
                    nc.vector.tensor_scalar_mul(ysb, py[:, 0:D], rec)
                    row = (g * (f // P) + it) * P
                    nc.sync.dma_start(out=y[row : row + P, :], in_=ysb)
    nc.compile()
    return nc


def make_in_maps(x, Wq, Wk, Wv, Wo):
    x = np.asarray(x, dtype=np.float32)
    Wq = np.asarray(Wq, dtype=np.float32)
    Wk = np.asarray(Wk, dtype=np.float32)
    Wv = np.asarray(Wv, dtype=np.float32)
    Wo = np.asarray(Wo, dtype=np.float32)
    xT = np.ascontiguousarray(x.T)
    in_maps = []
    for h in range(HEADS):
        sl = slice(h * D, (h + 1) * D)
        woT = np.zeros((Da, Da), np.float32)
        woT[:D, :D] = Wo[:, sl].T
        woT[D, D] = 1.0
        in_maps.append(
            {
                "xT": xT,
                "wqT": np.ascontiguousarray(Wq[sl].T),
                "wkT": np.ascontiguousarray(Wk[sl].T),
                "wvT": np.ascontiguousarray(Wv[sl].T),
                "woT": woT,
            }
        )
    return in_maps


_CACHE = {}


def run_on_hw(x, Wq, Wk, Wv, Wo, bo, trace=False):
    from concourse.bass_utils import run_bass_kernel_spmd

    if "nc" not in _CACHE:
        _CACHE["nc"] = build_nc()
    nc = _CACHE["nc"]
    in_maps = make_in_maps(x, Wq, Wk, Wv, Wo)
    res = run_bass_kernel_spmd(nc, in_maps, list(range(HEADS)), trace=trace)
    out = np.zeros((N, D), np.float32)
    for r in res.results:
        out += r["y"]
    out += np.asarray(bo, dtype=np.float32)[None, :]
    return out, res


def kernel(x, Wq, Wk, Wv, Wo, bo):
    out, _ = run_on_hw(x, Wq, Wk, Wv, Wo, bo)
    return out



# revision 2
# speedup vs baseline: 2.1120x; 2.1120x over previous
"""Graphformer layer (full multi-head attention) on 8 trn2 NeuronCores.

Sharding: one head per core (tensor parallel over the 8 heads).
Each core computes, for its head h:
    Q_h = x Wq_h^T, K_h = x Wk_h^T, V_h = x Wv_h^T          (4096, 64)
    S_h = Q_h K_h^T / 8;  P_h = softmax(S_h)                 (4096, 4096)
    y_core = (P_h V_h) Wo_h^T                                (4096, 64)
Full output = sum over cores + bo.

Perf formulation (all matmuls bf16: 1 PE cycle/moving column vs 4 for
fp32):
  - host passes x^T and the weights pre-transposed and pre-cast to bf16.
  - scores are computed transposed (S^T tiles: keys on partitions,
    queries on the free dim); softmax denominators come from an appended
    ones column on V (row 64 of the O^T accumulator).
  - exp() without max-subtraction: |S/8| < ~3 for these inputs.
  - exp is split across two engines: most tiles on the scalar (ACT)
    engine as exact Exp activations writing bf16; a fraction on the
    vector engine (DVE) via a Schraudolph bit-trick: bf16 bits of e^s
    are ~ trunc(a*s + b), computed by one tensor_scalar (mult+add) with
    int16 output aliasing the bf16 es tile. b comes in via DRAM so it
    can be recalibrated without recompiling.
  - normalization by the softmax denominator folds into the final output
    projection via an augmented (65,65) Wo^T with a 1 in the corner:
    column 64 of the Y tile is the per-row denominator.
"""

from contextlib import ExitStack

import numpy as np
import ml_dtypes

import concourse.bass as bass
import concourse.bacc as bacc
import concourse.mybir as mybir
from concourse.tile import TileContext

N = 4096
C = 512  # input feature dim
D = 64  # head dim
Da = D + 1  # head dim + denominator column
HEADS = 8
P = 128
F32 = mybir.dt.float32
BF16 = mybir.dt.bfloat16
I16 = mybir.dt.int16

# Schraudolph exp->bf16-bits constants: bits = trunc(A_SCHR * S + b)
# where s = S/8 (the 1/8 is folded into A_SCHR). b is a runtime input.
A_SCHR = (128.0 / float(np.log(2.0))) / 8.0
B_SCHR_TRUNC = 16250.89  # optimal if the f32->i16 convert truncates
B_SCHR_ROUND = 16250.39  # optimal if it rounds to nearest

# Fraction of key tiles whose exp runs on DVE instead of ACT.
DVE_PERIOD = 4  # every 4th tile -> DVE (25%)


def build_nc(n=N, f=1024):
    """Build the single-core SPMD program. n = sequence length, f = query
    group width (f*4 bytes*2 buffers of PSUM for scores)."""
    nt = n // P  # number of key/value tiles
    ct = C // P  # contraction tiles for projections
    g_count = n // f  # query groups

    nc = bacc.Bacc()
    xT = nc.declare_dram_parameter("xT", [C, n], BF16, isOutput=False)
    wqT = nc.declare_dram_parameter("wqT", [C, D], BF16, isOutput=False)
    wkT = nc.declare_dram_parameter("wkT", [C, D], BF16, isOutput=False)
    wvT = nc.declare_dram_parameter("wvT", [C, D], BF16, isOutput=False)
    woT = nc.declare_dram_parameter("woT", [Da, Da], BF16, isOutput=False)
    bsc = nc.declare_dram_parameter("bsc", [P, 1], F32, isOutput=False)
    y = nc.declare_dram_parameter("y", [n, D], F32, isOutput=True)

    with TileContext(nc) as tc, ExitStack() as ctx:
        const = ctx.enter_context(tc.tile_pool(name="const", bufs=1))
        sb = ctx.enter_context(tc.tile_pool(name="sb", bufs=1))
        es_pool = ctx.enter_context(tc.tile_pool(name="es", bufs=3))
        ot_pool = ctx.enter_context(tc.tile_pool(name="ot", bufs=2))
        y_pool = ctx.enter_context(tc.tile_pool(name="yp", bufs=4))

        # ---- load inputs
        xt = []
        for c in range(ct):
            t = sb.tile([P, n], BF16, tag=f"xt{c}")
            nc.sync.dma_start(out=t, in_=xT[c * P : (c + 1) * P, :])
            xt.append(t)
        w_sb = {}
        for name, dram in (("q", wqT), ("k", wkT), ("v", wvT)):
            t = const.tile([P, ct, D], BF16, tag=f"w{name}")
            for c in range(ct):
                nc.sync.dma_start(out=t[:, c, :], in_=dram[c * P : (c + 1) * P, :])
            w_sb[name] = t
        wo_sb = const.tile([Da, Da], BF16, tag="wo")
        nc.sync.dma_start(out=wo_sb, in_=woT[:, :])
        b_sb = const.tile([P, 1], F32, tag="bsc")
        nc.sync.dma_start(out=b_sb, in_=bsc[:, :])

        # ---- projections
        qT = sb.tile([D, n], BF16, tag="qT")
        kT = sb.tile([D, n], BF16, tag="kT")
        v_sb = sb.tile([P, nt, Da], BF16, tag="v")
        with tc.tile_pool(name="psP", bufs=4, space="PSUM") as psP:
            for chunk in range(n // 512):
                for dst, w in ((qT, w_sb["q"]), (kT, w_sb["k"])):
                    pp = psP.tile([D, 512], F32, tag="pqk")
                    for c in range(ct):
                        nc.tensor.matmul(
                            pp,
                            w[:, c, :],
                            xt[c][:, chunk * 512 : (chunk + 1) * 512],
                            start=(c == 0),
                            stop=(c == ct - 1),
                        )
                    # ACT does q/k copies (it is otherwise idle here)
                    nc.scalar.copy(
                        out=dst[:, chunk * 512 : (chunk + 1) * 512], in_=pp
                    )
            nc.vector.memset(v_sb[:, :, D:Da], 1.0)
            for mt in range(nt):
                pv = psP.tile([P, D], F32, tag="pv")
                for c in range(ct):
                    nc.tensor.matmul(
                        pv,
                        xt[c][:, mt * P : (mt + 1) * P],
                        w_sb["v"][:, c, :],
                        start=(c == 0),
                        stop=(c == ct - 1),
                    )
                nc.vector.tensor_copy(out=v_sb[:, mt, 0:D], in_=pv)

        # ---- attention + output projection, in query groups of f
        with (
            tc.tile_pool(name="psS", bufs=2, space="PSUM") as ps_s,
            tc.tile_pool(name="psO", bufs=1, space="PSUM") as ps_o,
            tc.tile_pool(name="psY", bufs=2, space="PSUM") as ps_y,
        ):
            for g in range(g_count):
                po = ps_o.tile([Da, f], F32, tag="O")
                for mt in range(nt):
                    ss = ps_s.tile([P, f], F32, tag="S")
                    for fc in range(f // 512):
                        nc.tensor.matmul(
                            ss[:, fc * 512 : (fc + 1) * 512],
                            kT[:, mt * P : (mt + 1) * P],
                            qT[:, g * f + fc * 512 : g * f + (fc + 1) * 512],
                            start=True,
                            stop=True,
                        )
                    es = es_pool.tile([P, f], BF16, tag="es")
                    if mt % DVE_PERIOD == DVE_PERIOD - 1:
                        # Schraudolph exp on DVE: bf16 bits = trunc(a*S + b)
                        nc.vector.tensor_scalar(
                            es.bitcast(I16),
                            ss,
                            A_SCHR,
                            b_sb[:, 0:1],
                            mybir.AluOpType.mult,
                            mybir.AluOpType.add,
                        )
                    else:
                        nc.scalar.activation(
                            out=es,
                            in_=ss,
                            func=mybir.ActivationFunctionType.Exp,
                            scale=0.125,
                        )
                    for fc in range(f // 512):
                        nc.tensor.matmul(
                            po[:, fc * 512 : (fc + 1) * 512],
                            v_sb[:, mt, :],
                            es[:, fc * 512 : (fc + 1) * 512],
                            start=(mt == 0),
                            stop=(mt == nt - 1),
                        )
                ot = ot_pool.tile([Da, f], BF16, tag="ot")
                nc.vector.tensor_copy(out=ot, in_=po)
                for it in range(f // P):
                    py = ps_y.tile([P, Da], F32, tag="Y")
                    nc.tensor.matmul(
                        py,
                        ot[:, it * P : (it + 1) * P],
                        wo_sb,
                        start=True,
                        stop=True,
                    )
                    rec = y_pool.tile([P, 1], F32, tag="rec")
                    nc.vector.reciprocal(rec, py[:, D:Da])
                    ysb = y_pool.tile([P, D], F32, tag="ysb")
                    nc.vector.tensor_scalar_mul(ysb, py[:, 0:D], rec)
                    row = (g * (f // P) + it) * P
                    nc.sync.dma_start(out=y[row : row + P, :], in_=ysb)
    nc.compile()
    return nc


def make_in_maps(x, Wq, Wk, Wv, Wo, b_schr=B_SCHR_TRUNC):
    bf = ml_dtypes.bfloat16
    x = np.asarray(x, dtype=np.float32)
    Wq = np.asarray(Wq, dtype=np.float32)
    Wk = np.asarray(Wk, dtype=np.float32)
    Wv = np.asarray(Wv, dtype=np.float32)
    Wo = np.asarray(Wo, dtype=np.float32)
    xT = np.ascontiguousarray(x.T).astype(bf)
    bsc = np.full((P, 1), b_schr, np.float32)
    in_maps = []
    for h in range(HEADS):
        sl = slice(h * D, (h + 1) * D)
        woT = np.zeros((Da, Da), np.float32)
        woT[:D, :D] = Wo[:, sl].T
        woT[D, D] = 1.0
        in_maps.append(
            {
                "xT": xT,
                "wqT": np.ascontiguousarray(Wq[sl].T).astype(bf),
                "wkT": np.ascontiguousarray(Wk[sl].T).astype(bf),
                "wvT": np.ascontiguousarray(Wv[sl].T).astype(bf),
                "woT": woT.astype(bf),
                "bsc": bsc,
            }
        )
    return in_maps


_CACHE = {}


def run_on_hw(x, Wq, Wk, Wv, Wo, bo, trace=False, b_schr=B_SCHR_TRUNC):
    from concourse.bass_utils import run_bass_kernel_spmd

    if "nc" not in _CACHE:
        _CACHE["nc"] = build_nc()
    nc = _CACHE["nc"]
    in_maps = make_in_maps(x, Wq, Wk, Wv, Wo, b_schr=b_schr)
    res = run_bass_kernel_spmd(nc, in_maps, list(range(HEADS)), trace=trace)
    out = np.zeros((N, D), np.float32)
    for r in res.results:
        out += r["y"]
    out += np.asarray(bo, dtype=np.float32)[None, :]
    return out, res


def kernel(x, Wq, Wk, Wv, Wo, bo):
    out, _ = run_on_hw(x, Wq, Wk, Wv, Wo, bo)
    return out
